# revision 3
# baseline (speedup 1.0000x reference)
"""Trainium2 Bass kernel for the Cut+Balance loss.

loss = sum_i numer_i / Gamma_i + sum_i (colsum(Y)_i - N/G)^2
  where B = Y^T A  (G x N),
        Gamma_i = sum_m B[i, m]
        numer_i = sum_m B[i, m] * (1 - Y[m, i]) = Gamma_i - sum_m B[i,m] Y[m,i]

Strategy (8 NeuronCores, row-sharded A):
  - Each core owns 2048 rows of A (128 MB) and computes the local
    B_c = Yl^T A_c contribution entirely in PSUM using fp32r matmuls
    (full-rate fp32 streaming; PSUM accumulates fp32). fp32r outputs
    must sit at PSUM partition offset 0, so the N=16384 columns are
    processed in four quarter-passes of 8 column-tiles x 512 (one psum
    bank each), accumulated over the 16 row-blocks of 128 rows.
  - A DMA covers one row-block x 4096 cols = 2 MB of 16 KB-contiguous
    rows -> near-peak HBM bandwidth (roofline ~358 GB/s/core ~ 375 us).
  - Y^T is loaded as a 16-partition tile (1 MB, no zero padding).
  - Per bank, ScalarE reduces psum rows to Gamma partials (activation
    Copy with accum_out) while VectorE computes the
    sum_m B[i,m] Y[m,i] partials with a fused tensor_tensor_reduce —
    the two engines run in parallel, halving the exposed tail chain.
  - Host sums the tiny per-core partials and adds the Y-only balance
    term.
"""

import sys

if "/opt/trn_rl_repo" not in sys.path:
    sys.path.insert(0, "/opt/trn_rl_repo")

import numpy as np

N = 16384
G = 16
NC = 8
R = N // NC            # 2048 rows of A per core
KT = R // 128          # 16 row-blocks per core
QP = 4                 # column quarter-passes
CN = N // QP           # 4096 columns per pass
JT = CN // 512         # 8 column tiles of 512 per pass (one psum bank each)

_NC_CACHE = None
last_results = None    # BassKernelResults of the most recent run


def _build():
    import concourse.mybir as mybir
    from concourse.bacc import Bacc
    from concourse.bass import MemorySpace, ds
    from concourse.tile import TileContext

    f32 = mybir.dt.float32
    f32r = mybir.dt.float32r

    nc = Bacc(trn_type="TRN2")
    a_d = nc.declare_dram_parameter("A", [R, N], f32r, isOutput=False)
    yl_d = nc.declare_dram_parameter("Ylp", [128, KT, G], f32r, isOutput=False)
    yt_d = nc.declare_dram_parameter("YTp", [G, QP, JT, 512], f32, isOutput=False)
    out_d = nc.declare_dram_parameter("out", [G, 2 * QP * JT], f32, isOutput=True)

    with TileContext(nc) as tc:
        with (
            tc.tile_pool(name="const", bufs=1) as cpool,
            tc.tile_pool(name="abuf", bufs=6) as apool,
            tc.tile_pool(name="scr", bufs=2) as spool,
            tc.tile_pool(name="sact", bufs=2) as actpool,
            tc.tile_pool(name="psum", bufs=1, space=MemorySpace.PSUM) as ppool,
        ):
            yl = cpool.tile([128, KT, G], f32r)
            nc.sync.dma_start(out=yl, in_=yl_d[:])
            # Y^T packed on 16 partitions; pass indexed on the free dim so
            # all DVE operands share partition base 0 (ISA requirement).
            yt = cpool.tile([G, QP, JT, 512], f32)
            gs = cpool.tile([G, 2 * QP * JT], f32)

            psums = [
                ppool.tile([128, 512], f32, name=f"psum{b}") for b in range(JT)
            ]

            yt_loaded = False
            for p in range(QP):
                for k in range(KT):
                    ak = apool.tile([128, CN], f32r)
                    last_tile = p == QP - 1 and k == KT - 1
                    if last_tile:
                        # Split the final tile's DMA so banks 0-3 can
                        # start their matmuls/reductions ~3 us earlier.
                        half = CN // 2
                        nc.sync.dma_start(
                            out=ak[:, ds(0, half)],
                            in_=a_d[ds(k * 128, 128), ds(p * CN, half)],
                        )
                        nc.sync.dma_start(
                            out=ak[:, ds(half, half)],
                            in_=a_d[ds(k * 128, 128), ds(p * CN + half, half)],
                        )
                    else:
                        nc.sync.dma_start(
                            out=ak, in_=a_d[ds(k * 128, 128), ds(p * CN, CN)]
                        )
                    if not yt_loaded:
                        # Issued after the first A tile so the A stream
                        # starts as early as possible; needed only by the
                        # first reductions ~100 us in.
                        nc.sync.dma_start(out=yt, in_=yt_d[:])
                        yt_loaded = True
                    for jj in range(JT):
                        nc.tensor.matmul(
                            psums[jj][ds(0, G), :],
                            yl[:, k, :],
                            ak[:, ds(512 * jj, 512)],
                            start=(k == 0),
                            stop=(k == KT - 1),
                        )

                # Per-bank reductions of psum rows 0..15: Gamma partial on
                # ScalarE, Y-weighted partial on VectorE (parallel chains).
                for jj in range(JT):
                    scr_a = actpool.tile([G, 512], f32)
                    nc.scalar.activation(
                        out=scr_a,
                        in_=psums[jj][ds(0, G), :],
                        func=mybir.ActivationFunctionType.Copy,
                        accum_out=gs[ds(0, G), ds(p * JT + jj, 1)],
                    )
                    # (tensor_tensor_reduce faults at runtime on HW here;
                    # use separate multiply + reduce instead)
                    scr_v = spool.tile([G, 512], f32)
                    nc.vector.tensor_mul(
                        scr_v,
                        psums[jj][ds(0, G), :],
                        yt[ds(0, G), p, jj, :],
                    )
                    nc.vector.tensor_reduce(
                        out=gs[ds(0, G), ds(QP * JT + p * JT + jj, 1)],
                        in_=scr_v,
                        axis=mybir.AxisListType.X,
                        op=mybir.AluOpType.add,
                    )

            nc.sync.dma_start(out=out_d[:], in_=gs)

    nc.finalize()  # Bacc: runs wait-splitting (generate_event_semaphores) + reg alloc
    return nc


def _get_nc():
    global _NC_CACHE
    if _NC_CACHE is None:
        _NC_CACHE = _build()
    return _NC_CACHE


def _pack_inputs(Y, A):
    """Host-side packed layouts so the device does zero reshuffling."""
    # Ylp[c][p, k, i] = Y[c*R + k*128 + p, i]  (matmul lhsT per row-block)
    ylp = Y.reshape(NC, KT, 128, G).transpose(0, 2, 1, 3).copy()
    # YTp[i, p, jj, f] = Y[(p*JT + jj)*512 + f, i]
    ytp = np.ascontiguousarray(Y.reshape(QP, JT, 512, G).transpose(3, 0, 1, 2))
    in_maps = [
        {"A": np.ascontiguousarray(A[c * R : (c + 1) * R]), "Ylp": ylp[c], "YTp": ytp}
        for c in range(NC)
    ]
    return in_maps


def kernel(Y, A, _trace=False, _trace_kwargs=None):
    global last_results
    Y = np.asarray(Y, dtype=np.float32)
    A = np.asarray(A, dtype=np.float32)
    assert Y.shape == (N, G) and A.shape == (N, N)

    from concourse.bass_utils import run_bass_kernel_spmd

    in_maps = _pack_inputs(Y, A)
    res = run_bass_kernel_spmd(
        _get_nc(),
        in_maps,
        core_ids=list(range(NC)),
        trace=_trace,
        **(_trace_kwargs or {}),
    )
    last_results = res

    g_total = np.zeros(G, dtype=np.float64)
    s_total = np.zeros(G, dtype=np.float64)
    for c in range(NC):
        o = np.asarray(res.results[c]["out"], dtype=np.float64)  # [G, 2*QP*JT]
        g_total += o[:, : QP * JT].sum(axis=1)
        s_total += o[:, QP * JT :].sum(axis=1)

    gamma = g_total
    numer = gamma - s_total
    cut = float(np.sum(numer / gamma))
    col = Y.sum(axis=0, dtype=np.float64)
    balance = float(np.sum((col - N / G) ** 2))
    return np.float32(cut + balance)


# revision 7
# speedup vs baseline: 1.0346x; 1.0346x over previous
"""Trainium2 Bass kernel for the Cut+Balance loss.

loss = sum_i numer_i / Gamma_i + sum_i (colsum(Y)_i - N/G)^2
  where B = Y^T A  (G x N),
        Gamma_i = sum_m B[i, m]
        numer_i = sum_m B[i, m] * (1 - Y[m, i]) = Gamma_i - sum_m B[i,m] Y[m,i]

Strategy (8 NeuronCores, row-sharded A):
  - Each core owns 2048 rows of A (128 MB) and computes the local
    B_c = Yl^T A_c contribution entirely in PSUM using fp32r matmuls
    (full-rate fp32 streaming; PSUM accumulates fp32). fp32r outputs
    must sit at PSUM partition offset 0, so the N=16384 columns are
    processed in four quarter-passes of 8 column-tiles x 512 (one psum
    bank each), accumulated over the 16 row-blocks of 128 rows.
  - A DMA covers one row-block x 4096 cols = 2 MB of 16 KB-contiguous
    rows -> near-peak HBM bandwidth (roofline ~358 GB/s/core ~ 375 us).
  - Y^T is loaded as a 16-partition tile (1 MB, no zero padding).
  - Per bank, ScalarE reduces psum rows to Gamma partials (activation
    Copy with accum_out) while VectorE computes the
    sum_m B[i,m] Y[m,i] partials with a fused tensor_tensor_reduce —
    the two engines run in parallel, halving the exposed tail chain.
  - Host sums the tiny per-core partials and adds the Y-only balance
    term.
"""

import sys

if "/opt/trn_rl_repo" not in sys.path:
    sys.path.insert(0, "/opt/trn_rl_repo")

import numpy as np

N = 16384
G = 16
NC = 8
R = N // NC            # 2048 rows of A per core
KT = R // 128          # 16 row-blocks per core
QP = 4                 # column quarter-passes
CN = N // QP           # 4096 columns per pass
JT = CN // 512         # 8 column tiles of 512 per pass (one psum bank each)

_NC_CACHE = None
last_results = None    # BassKernelResults of the most recent run


def _build():
    import concourse.mybir as mybir
    from concourse.bacc import Bacc
    from concourse.bass import MemorySpace, ds
    from concourse.tile import TileContext

    f32 = mybir.dt.float32
    f32r = mybir.dt.float32r

    nc = Bacc(trn_type="TRN2")
    a_d = nc.declare_dram_parameter("A", [R, N], f32r, isOutput=False)
    yl_d = nc.declare_dram_parameter("Ylp", [128, KT, G], f32r, isOutput=False)
    yt_d = nc.declare_dram_parameter("YTp", [G, QP, JT, 512], f32, isOutput=False)
    out_d = nc.declare_dram_parameter("out", [G, 2 * QP * JT], f32, isOutput=True)

    with TileContext(nc) as tc:
        with (
            tc.tile_pool(name="const", bufs=1) as cpool,
            tc.tile_pool(name="abuf", bufs=6) as apool,
            tc.tile_pool(name="scr", bufs=2) as spool,
            tc.tile_pool(name="sact", bufs=2) as actpool,
            tc.tile_pool(name="psum", bufs=1, space=MemorySpace.PSUM) as ppool,
        ):
            yl = cpool.tile([128, KT, G], f32r)
            nc.sync.dma_start(out=yl, in_=yl_d[:])
            # Y^T packed on 16 partitions; pass indexed on the free dim so
            # all DVE operands share partition base 0 (ISA requirement).
            yt = cpool.tile([G, QP, JT, 512], f32)
            # Separate tiles per producing engine so Tile's write tracking
            # never serializes the ScalarE and VectorE reduction chains.
            gs_g = cpool.tile([G, QP * JT], f32)
            gs_s = cpool.tile([G, QP * JT], f32)

            psums = [
                ppool.tile([128, 512], f32, name=f"psum{b}") for b in range(JT)
            ]

            yt_loaded = False
            for p in range(QP):
                for k in range(KT):
                    ak = apool.tile([128, CN], f32r)
                    last_tile = p == QP - 1 and k == KT - 1
                    if last_tile:
                        # Stream the final row-block in 1024-col chunks
                        # with per-chunk matmuls so most banks' reductions
                        # overlap the remaining DMA instead of the tail.
                        for c in range(4):
                            nc.sync.dma_start(
                                out=ak[:, ds(1024 * c, 1024)],
                                in_=a_d[
                                    ds(k * 128, 128), ds(p * CN + 1024 * c, 1024)
                                ],
                            )
                            for jj in (2 * c, 2 * c + 1):
                                nc.tensor.matmul(
                                    psums[jj][ds(0, G), :],
                                    yl[:, k, :],
                                    ak[:, ds(512 * jj, 512)],
                                    start=(k == 0),
                                    stop=(k == KT - 1),
                                )
                        continue_matmuls = False
                    else:
                        nc.sync.dma_start(
                            out=ak, in_=a_d[ds(k * 128, 128), ds(p * CN, CN)]
                        )
                        continue_matmuls = True
                    if not yt_loaded:
                        # Issued after the first A tile so the A stream
                        # starts as early as possible; needed only by the
                        # first reductions ~100 us in.
                        nc.sync.dma_start(out=yt, in_=yt_d[:])
                        yt_loaded = True
                    if continue_matmuls:
                        for jj in range(JT):
                            nc.tensor.matmul(
                                psums[jj][ds(0, G), :],
                                yl[:, k, :],
                                ak[:, ds(512 * jj, 512)],
                                start=(k == 0),
                                stop=(k == KT - 1),
                            )

                # Per-bank reductions of psum rows 0..15: Gamma partial on
                # ScalarE, Y-weighted partial on VectorE (parallel chains).
                for jj in range(JT):
                    scr_a = actpool.tile([G, 512], f32)
                    nc.scalar.activation(
                        out=scr_a,
                        in_=psums[jj][ds(0, G), :],
                        func=mybir.ActivationFunctionType.Copy,
                        accum_out=gs_g[ds(0, G), ds(p * JT + jj, 1)],
                    )
                    # (tensor_tensor_reduce faults at runtime on HW here;
                    # use separate multiply + reduce instead)
                    scr_v = spool.tile([G, 512], f32)
                    nc.vector.tensor_mul(
                        scr_v,
                        psums[jj][ds(0, G), :],
                        yt[ds(0, G), p, jj, :],
                    )
                    nc.vector.tensor_reduce(
                        out=gs_s[ds(0, G), ds(p * JT + jj, 1)],
                        in_=scr_v,
                        axis=mybir.AxisListType.X,
                        op=mybir.AluOpType.add,
                    )

            nc.sync.dma_start(out=out_d[:, ds(0, QP * JT)], in_=gs_g)
            nc.sync.dma_start(out=out_d[:, ds(QP * JT, QP * JT)], in_=gs_s)

    nc.finalize()  # Bacc: runs wait-splitting (generate_event_semaphores) + reg alloc
    return nc


def _get_nc():
    global _NC_CACHE
    if _NC_CACHE is None:
        _NC_CACHE = _build()
    return _NC_CACHE


def _pack_inputs(Y, A):
    """Host-side packed layouts so the device does zero reshuffling."""
    # Ylp[c][p, k, i] = Y[c*R + k*128 + p, i]  (matmul lhsT per row-block)
    ylp = Y.reshape(NC, KT, 128, G).transpose(0, 2, 1, 3).copy()
    # YTp[i, p, jj, f] = Y[(p*JT + jj)*512 + f, i]
    ytp = np.ascontiguousarray(Y.reshape(QP, JT, 512, G).transpose(3, 0, 1, 2))
    in_maps = [
        {"A": np.ascontiguousarray(A[c * R : (c + 1) * R]), "Ylp": ylp[c], "YTp": ytp}
        for c in range(NC)
    ]
    return in_maps


def kernel(Y, A, _trace=False, _trace_kwargs=None):
    global last_results
    Y = np.asarray(Y, dtype=np.float32)
    A = np.asarray(A, dtype=np.float32)
    assert Y.shape == (N, G) and A.shape == (N, N)

    from concourse.bass_utils import run_bass_kernel_spmd

    in_maps = _pack_inputs(Y, A)
    res = run_bass_kernel_spmd(
        _get_nc(),
        in_maps,
        core_ids=list(range(NC)),
        trace=_trace,
        **(_trace_kwargs or {}),
    )
    last_results = res

    g_total = np.zeros(G, dtype=np.float64)
    s_total = np.zeros(G, dtype=np.float64)
    for c in range(NC):
        o = np.asarray(res.results[c]["out"], dtype=np.float64)  # [G, 2*QP*JT]
        g_total += o[:, : QP * JT].sum(axis=1)
        s_total += o[:, QP * JT :].sum(axis=1)

    gamma = g_total
    numer = gamma - s_total
    cut = float(np.sum(numer / gamma))
    col = Y.sum(axis=0, dtype=np.float64)
    balance = float(np.sum((col - N / G) ** 2))
    return np.float32(cut + balance)


# revision 16
# speedup vs baseline: 1.1959x; 1.1560x over previous
"""Trainium2 Bass kernel for the Cut+Balance loss.

loss = sum_i numer_i / Gamma_i + sum_i (colsum(Y)_i - N/G)^2
  where B = Y^T A  (G x N),
        Gamma_i = sum_m B[i, m]
        numer_i = sum_m B[i, m] * (1 - Y[m, i]) = Gamma_i - sum_m B[i,m] Y[m,i]

Strategy (8 NeuronCores, row-sharded A):
  - Each core owns 2048 rows of A (128 MB) and computes the local
    B_c = Yl^T A_c contribution entirely in PSUM using fp32r matmuls
    (full-rate fp32 streaming; PSUM accumulates fp32). fp32r outputs
    must sit at PSUM partition offset 0, so the N=16384 columns are
    processed in four quarter-passes of 8 column-tiles x 512 (one psum
    bank each), accumulated over the 16 row-blocks of 128 rows.
  - A DMA covers one row-block x 4096 cols = 2 MB of 16 KB-contiguous
    rows -> near-peak HBM bandwidth (roofline ~358 GB/s/core ~ 375 us).
  - Y^T is loaded as a 16-partition tile (1 MB, no zero padding).
  - Per bank, ScalarE reduces psum rows to Gamma partials (activation
    Copy with accum_out) while VectorE computes the
    sum_m B[i,m] Y[m,i] partials with a fused tensor_tensor_reduce —
    the two engines run in parallel, halving the exposed tail chain.
  - Host sums the tiny per-core partials and adds the Y-only balance
    term.
"""

import sys

if "/opt/trn_rl_repo" not in sys.path:
    sys.path.insert(0, "/opt/trn_rl_repo")

import numpy as np

N = 16384
G = 16
NC = 8
R = N // NC            # 2048 rows of A per core
KT = R // 128          # 16 row-blocks per core
CN = 4096              # max columns per pass (A-tile buffer width)
PASSES = [8, 8, 8, 6, 2]  # psum banks (512-col tiles) per column pass

_NC_CACHE = None
last_results = None    # BassKernelResults of the most recent run


def _build():
    import concourse.mybir as mybir
    from concourse.bacc import Bacc
    from concourse.bass import MemorySpace, ds
    from concourse.tile import TileContext

    f32 = mybir.dt.float32
    f32r = mybir.dt.float32r

    TC = N // 512  # 32 column tiles of 512 overall

    nc = Bacc(trn_type="TRN2")
    a_d = nc.declare_dram_parameter("A", [R, N], f32r, isOutput=False)
    yl_d = nc.declare_dram_parameter("Ylp", [128, KT, G], f32r, isOutput=False)
    yt_d = nc.declare_dram_parameter("YTp", [G, TC, 512], f32, isOutput=False)
    out_d = nc.declare_dram_parameter("out", [G, 2 * TC], f32, isOutput=True)

    with TileContext(nc) as tc:
        with (
            tc.tile_pool(name="const", bufs=1) as cpool,
            tc.tile_pool(name="abuf", bufs=6) as apool,
            tc.tile_pool(name="scr", bufs=2) as spool,
            tc.tile_pool(name="sact", bufs=2) as actpool,
            tc.tile_pool(name="psum", bufs=1, space=MemorySpace.PSUM) as ppool,
        ):
            yl = cpool.tile([128, KT, G], f32r)
            # Y^T packed on 16 partitions; column-tile indexed on the free
            # dim so all DVE operands share partition base 0 (ISA req).
            yt = cpool.tile([G, TC, 512], f32)
            # Separate tiles per producing engine so Tile's write tracking
            # never serializes the ScalarE and VectorE reduction chains.
            gs_g = cpool.tile([G, TC], f32)
            gs_s = cpool.tile([G, TC], f32)

            psums = [ppool.tile([128, 512], f32, name=f"psum{b}") for b in range(8)]

            # Uneven column passes: the final pass covers only 2 of the 32
            # column tiles, so the exposed end-of-kernel reduction chain is
            # 2 banks instead of 8 (the wider passes' chains overlap DMA).
            first = True
            t0 = 0  # global column-tile index of current pass start
            for nb in PASSES:
                CNp = nb * 512
                for k in range(KT):
                    ak = apool.tile([128, CN], f32r)
                    last_block = t0 + nb == TC and k == KT - 1
                    if last_block:
                        # Stream the final row-block one bank at a time so
                        # each bank's reductions overlap the remaining DMA.
                        for jj in range(nb):
                            nc.sync.dma_start(
                                out=ak[:, ds(512 * jj, 512)],
                                in_=a_d[
                                    ds(k * 128, 128),
                                    ds((t0 + jj) * 512, 512),
                                ],
                            )
                            nc.tensor.matmul(
                                psums[jj][ds(0, G), :],
                                yl[:, k, :],
                                ak[:, ds(512 * jj, 512)],
                                start=(k == 0),
                                stop=(k == KT - 1),
                            )
                    else:
                        nc.sync.dma_start(
                            out=ak[:, ds(0, CNp)],
                            in_=a_d[ds(k * 128, 128), ds(t0 * 512, CNp)],
                        )
                        if first:
                            # Y loads issued behind the first A tile so the
                            # A stream starts immediately; yl is needed by
                            # the first matmul, yt by the first reductions.
                            nc.sync.dma_start(out=yl, in_=yl_d[:])
                            nc.sync.dma_start(out=yt, in_=yt_d[:])
                            first = False
                        for jj in range(nb):
                            nc.tensor.matmul(
                                psums[jj][ds(0, G), :],
                                yl[:, k, :],
                                ak[:, ds(512 * jj, 512)],
                                start=(k == 0),
                                stop=(k == KT - 1),
                            )

                # Per-bank reductions of psum rows 0..15: a single psum
                # read per bank by ScalarE (copy to SBUF + Gamma partial
                # via accum_out); VectorE then multiplies by Y^T and
                # reduces purely from SBUF, so the two engines never
                # contend on the psum bank. (tensor_tensor_reduce faults
                # at runtime on HW; use separate multiply + reduce.)
                for jj in range(nb):
                    t = t0 + jj
                    scr_c = actpool.tile([G, 512], f32)
                    nc.scalar.activation(
                        out=scr_c,
                        in_=psums[jj][ds(0, G), :],
                        func=mybir.ActivationFunctionType.Copy,
                        accum_out=gs_g[ds(0, G), ds(t, 1)],
                    )
                    scr_v = spool.tile([G, 512], f32)
                    nc.vector.tensor_mul(
                        scr_v,
                        scr_c,
                        yt[ds(0, G), t, :],
                    )
                    nc.vector.tensor_reduce(
                        out=gs_s[ds(0, G), ds(t, 1)],
                        in_=scr_v,
                        axis=mybir.AxisListType.X,
                        op=mybir.AluOpType.add,
                    )
                t0 += nb

            nc.sync.dma_start(out=out_d[:, ds(0, TC)], in_=gs_g)
            nc.sync.dma_start(out=out_d[:, ds(TC, TC)], in_=gs_s)

    nc.finalize()  # Bacc: runs wait-splitting (generate_event_semaphores) + reg alloc
    return nc


def _get_nc():
    global _NC_CACHE
    if _NC_CACHE is None:
        _NC_CACHE = _build()
    return _NC_CACHE


def _pack_inputs(Y, A):
    """Host-side packed layouts so the device does zero reshuffling."""
    # Ylp[c][p, k, i] = Y[c*R + k*128 + p, i]  (matmul lhsT per row-block)
    ylp = Y.reshape(NC, KT, 128, G).transpose(0, 2, 1, 3).copy()
    # YTp[i, t, f] = Y[t*512 + f, i]
    ytp = np.ascontiguousarray(Y.reshape(N // 512, 512, G).transpose(2, 0, 1))
    in_maps = [
        {"A": np.ascontiguousarray(A[c * R : (c + 1) * R]), "Ylp": ylp[c], "YTp": ytp}
        for c in range(NC)
    ]
    return in_maps


def kernel(Y, A, _trace=False, _trace_kwargs=None):
    global last_results
    Y = np.asarray(Y, dtype=np.float32)
    A = np.asarray(A, dtype=np.float32)
    assert Y.shape == (N, G) and A.shape == (N, N)

    from concourse.bass_utils import run_bass_kernel_spmd

    in_maps = _pack_inputs(Y, A)
    res = run_bass_kernel_spmd(
        _get_nc(),
        in_maps,
        core_ids=list(range(NC)),
        trace=_trace,
        **(_trace_kwargs or {}),
    )
    last_results = res

    TC = N // 512
    g_total = np.zeros(G, dtype=np.float64)
    s_total = np.zeros(G, dtype=np.float64)
    for c in range(NC):
        o = np.asarray(res.results[c]["out"], dtype=np.float64)  # [G, 2*TC]
        g_total += o[:, :TC].sum(axis=1)
        s_total += o[:, TC:].sum(axis=1)

    gamma = g_total
    numer = gamma - s_total
    cut = float(np.sum(numer / gamma))
    col = Y.sum(axis=0, dtype=np.float64)
    balance = float(np.sum((col - N / G) ** 2))
    return np.float32(cut + balance)
